# revision 5
# baseline (speedup 1.0000x reference)
"""Bass/Trainium2 kernel for DropConnect (training path, Wstd != 0).

Z[b,o] = sum_i X[b,i] * W[i,o] * Werr[loc_id[b],i,o] + bias[o] * Berr[loc_id[b],o]

Strategy (8 NeuronCores, data-parallel over batch):
  - each core handles 16 samples; loc_id is known on the host at launch, so
    the per-sample Werr/Berr rows are gathered host-side while sharding and
    shipped per-core as plain contiguous inputs (the "all-gather of the
    needed rows" sharding choice) -- no on-device indirect DMA at all
  - slabs are shipped in bf16 (tolerance is 2e-2; measured end-to-end
    rel err ~3e-3), halving HBM traffic to ~8.4 MB/core
  - the slab stream is split across BOTH HWDGE rings (SP + Activation) so
    the 16 SDMA engines stay continuously fed; small inputs load first
  - slab x W elementwise products run on VectorE (2x_1P bf16 mode), with
    every 4th slab offloaded to the otherwise-idle GpSimd engine
  - TensorE contracts with X: per sample, 4 accumulating [128,1]x[128,512]
    matmuls into a [1,512] PSUM tile plus a 5th [16,1]x[16,512] eye-column
    matmul that adds the bias*Berr row
  - ScalarE copies each PSUM row into a [1,8192] staging tile; the output
    ships in 4 x 8KB DMAs so the store overlaps the tail of the compute
"""

import sys

sys.path.insert(0, "/opt/trn_rl_repo")

import ml_dtypes
import numpy as np

B, IN, OUT, POOL, NCORES = 128, 512, 512, 1000, 8
BL = B // NCORES  # samples per core
WT_COLS = 4 * OUT  # 2048: one macro-row = 4 input rows of W/Werr

BF16 = ml_dtypes.bfloat16

# slab -> chunk grouping: two 1-slab chunks first (fast first compute),
# then 2-slab chunks. chunk i alternates between the two HWDGE rings.
CHUNK_SLABS = [[0], [1], [2, 3], [4, 5], [6, 7], [8, 9], [10, 11], [12, 13], [14, 15]]
# slabs whose elementwise product runs on GpSimd instead of VectorE
POOL_SLABS = {2, 6, 10, 14}

_CACHE = {}


def _build():
    import concourse.mybir as mybir
    import concourse.tile as tile
    from concourse import bacc

    f32, bf16 = mybir.dt.float32, mybir.dt.bfloat16

    nc = bacc.Bacc("TRN2", debug=False)
    wd = nc.dram_tensor("WD", [128, BL * WT_COLS], bf16, kind="ExternalInput")
    wr = nc.dram_tensor("Wr", [128, WT_COLS], bf16, kind="ExternalInput")
    xt = nc.dram_tensor("Xt", [128, BL * 4], bf16, kind="ExternalInput")
    eye = nc.dram_tensor("Eye", [BL, BL], bf16, kind="ExternalInput")
    bias16 = nc.dram_tensor("bias16", [BL, OUT], f32, kind="ExternalInput")
    berr16 = nc.dram_tensor("berr16", [BL, OUT], f32, kind="ExternalInput")
    z = nc.dram_tensor("Z", [1, BL * OUT], f32, kind="ExternalOutput")

    rings = [nc.sync, nc.scalar]

    with tile.TileContext(nc) as tc:
        with (
            tc.tile_pool(name="const", bufs=1) as cpool,
            tc.tile_pool(name="wts", bufs=4) as wpool,
            tc.tile_pool(name="prod", bufs=3) as ptpool,
            tc.tile_pool(name="ps", bufs=8, space="PSUM") as ppool,
        ):
            # ring 0 (SP): slab chunk 0 first, then the small inputs;
            # ring 1 (ACT): slab chunks 1,3,5,... -- so the first slab, W,
            # bias inputs and the second slab all land within ~2us and the
            # engines never sit behind a big serialized stream.
            wt_tiles = {}

            def chunk_dma(ci):
                slabs = CHUNK_SLABS[ci]
                w = len(slabs) * WT_COLS
                # uniform tile size so the pool recycles cleanly
                t = wpool.tile([128, 2 * WT_COLS], bf16, tag="wt")
                ring = rings[1] if ci == len(CHUNK_SLABS) - 1 else rings[ci % 2]
                ring.dma_start(
                    t[:, :w], wd.ap()[:, slabs[0] * WT_COLS : slabs[0] * WT_COLS + w]
                )
                for si, s in enumerate(slabs):
                    wt_tiles[s] = (t, si)

            chunk_dma(0)  # ring0: slab 0
            chunk_dma(1)  # ring1: slab 1
            wr_sb = cpool.tile([128, WT_COLS], bf16)
            nc.sync.dma_start(wr_sb[:], wr.ap())
            xt_sb = cpool.tile([128, BL * 4], bf16)
            nc.sync.dma_start(xt_sb[:], xt.ap())
            eye_sb = cpool.tile([BL, BL], bf16)
            nc.sync.dma_start(eye_sb[:], eye.ap())
            bias_sb = cpool.tile([BL, OUT], f32)
            nc.sync.dma_start(bias_sb[:], bias16.ap())
            berr_sb = cpool.tile([BL, OUT], f32)
            nc.sync.dma_start(berr_sb[:], berr16.ap())
            for ci in range(2, len(CHUNK_SLABS)):
                chunk_dma(ci)

            memb_sb = cpool.tile([BL, OUT], bf16)
            nc.vector.tensor_mul(memb_sb[:], berr_sb[:], bias_sb[:])
            zstage = cpool.tile([1, BL * OUT], f32)

            for b in range(BL):
                wt, si = wt_tiles[b]
                pt = ptpool.tile([128, WT_COLS], bf16, tag="pt")
                mul_eng = nc.gpsimd if b in POOL_SLABS else nc.vector
                mul_eng.tensor_mul(
                    pt[:], wt[:, si * WT_COLS : (si + 1) * WT_COLS], wr_sb[:]
                )
                ps = ppool.tile([1, OUT], f32, tag="ps")
                for j in range(4):
                    nc.tensor.matmul(
                        out=ps[:],
                        lhsT=xt_sb[:, 4 * b + j : 4 * b + j + 1],
                        rhs=pt[:, j * OUT : (j + 1) * OUT],
                        start=(j == 0),
                        stop=False,
                    )
                nc.tensor.matmul(
                    out=ps[:],
                    lhsT=eye_sb[:, b : b + 1],
                    rhs=memb_sb[:],
                    start=False,
                    stop=True,
                )
                nc.scalar.copy(out=zstage[0:1, b * OUT : (b + 1) * OUT], in_=ps[:])
                if b % 4 == 3:
                    g = b // 4
                    nc.scalar.dma_start(
                        z.ap()[:, g * 4 * OUT : (g + 1) * 4 * OUT],
                        zstage[0:1, g * 4 * OUT : (g + 1) * 4 * OUT],
                    )

    nc.compile()
    return nc


def get_nc():
    if "nc" not in _CACHE:
        _CACHE["nc"] = _build()
    return _CACHE["nc"]


def make_in_maps(X, W, bias, Werr, Berr, loc_id):
    X = np.ascontiguousarray(np.asarray(X, dtype=np.float32))
    W = np.ascontiguousarray(np.asarray(W, dtype=np.float32))
    bias = np.ascontiguousarray(np.asarray(bias, dtype=np.float32))
    Werr = np.asarray(Werr, dtype=np.float32)
    Berr = np.asarray(Berr, dtype=np.float32)
    loc_id = np.asarray(loc_id, dtype=np.int32)

    wrb = np.ascontiguousarray(W.reshape(128, WT_COLS).astype(BF16))
    bias16 = np.ascontiguousarray(np.broadcast_to(bias[None, :], (BL, OUT)))
    eye16 = np.eye(BL, dtype=BF16)

    in_maps = []
    for c in range(NCORES):
        xc = X[c * BL : (c + 1) * BL]  # [BL, IN]
        locc = loc_id[c * BL : (c + 1) * BL]  # [BL]
        # slab b in columns [b*2048:(b+1)*2048]; partition p = in-rows 4p..4p+3
        wdc = np.ascontiguousarray(
            Werr[locc]
            .astype(BF16)
            .reshape(BL, 128, WT_COLS)
            .transpose(1, 0, 2)
            .reshape(128, BL * WT_COLS)
        )
        xtc = np.ascontiguousarray(
            xc.reshape(BL, 128, 4).transpose(1, 0, 2).reshape(128, BL * 4).astype(BF16)
        )
        in_maps.append(
            {
                "WD": wdc,
                "Wr": wrb,
                "Xt": xtc,
                "Eye": eye16,
                "bias16": bias16,
                "berr16": np.ascontiguousarray(Berr[locc]),
            }
        )
    return in_maps


def _reset_accelerator():
    import ctypes

    try:
        lib = ctypes.CDLL("/opt/axon/libaxon_pjrt.so")
        lib.axon_reset.restype = ctypes.c_int64
        lib.axon_reset()
    except Exception:
        pass


def kernel(X, W, bias, Werr, Berr, loc_id):
    from concourse.bass_utils import run_bass_kernel_spmd

    nc = get_nc()
    in_maps = make_in_maps(X, W, bias, Werr, Berr, loc_id)
    try:
        res = run_bass_kernel_spmd(nc, in_maps, core_ids=list(range(NCORES)))
    except Exception:
        # a wedged NeuronCore surfaces as an unrecoverable-device error;
        # reset the accelerator once and retry
        _reset_accelerator()
        res = run_bass_kernel_spmd(nc, in_maps, core_ids=list(range(NCORES)))
    out = np.concatenate(
        [res.results[c]["Z"].reshape(BL, OUT) for c in range(NCORES)], axis=0
    )
    return out


# revision 7
# speedup vs baseline: 1.2221x; 1.2221x over previous
"""Bass/Trainium2 kernel for DropConnect (training path, Wstd != 0).

Z[b,o] = sum_i X[b,i] * W[i,o] * Werr[loc_id[b],i,o] + bias[o] * Berr[loc_id[b],o]

Strategy (8 NeuronCores, data-parallel over batch):
  - each core handles 16 samples; loc_id is known on the host at launch, so
    the per-sample Werr/Berr rows are gathered host-side while sharding and
    shipped per-core as plain contiguous inputs (the "all-gather of the
    needed rows" sharding choice) -- no on-device indirect DMA at all
  - slabs are shipped in bf16 (tolerance is 2e-2; measured end-to-end
    rel err ~3e-3), halving HBM traffic to ~8.4 MB/core
  - the slab stream is split across BOTH HWDGE rings (SP + Activation) so
    the 16 SDMA engines stay continuously fed; small inputs load first
  - slab x W elementwise products run on VectorE (2x_1P bf16 mode), with
    every 4th slab offloaded to the otherwise-idle GpSimd engine
  - TensorE contracts with X: per sample, 4 accumulating [128,1]x[128,512]
    matmuls into a [1,512] PSUM tile plus a 5th [16,1]x[16,512] eye-column
    matmul that adds the bias*Berr row
  - ScalarE copies each PSUM row into a [1,8192] staging tile; the output
    ships in 4 x 8KB DMAs so the store overlaps the tail of the compute
"""

import sys

sys.path.insert(0, "/opt/trn_rl_repo")

import ml_dtypes
import numpy as np

B, IN, OUT, POOL, NCORES = 128, 512, 512, 1000, 8
BL = B // NCORES  # samples per core
WT_COLS = 4 * OUT  # 2048: one macro-row = 4 input rows of W/Werr

BF16 = ml_dtypes.bfloat16

# slab -> chunk grouping: two 1-slab chunks first (fast first compute),
# then 2-slab chunks. chunk i alternates between the two HWDGE rings.
CHUNK_SLABS = [[0], [1], [2, 3], [4, 5], [6, 7], [8, 9], [10, 11], [12, 13], [14, 15]]
# slabs whose elementwise product runs on GpSimd instead of VectorE
# (measured: Pool TT is ~4.4us/slab vs DVE 1.23us -- offload loses)
POOL_SLABS = set()

_CACHE = {}


def _build():
    import concourse.mybir as mybir
    import concourse.tile as tile
    from concourse import bacc

    f32, bf16 = mybir.dt.float32, mybir.dt.bfloat16

    nc = bacc.Bacc("TRN2", debug=False)
    wd = nc.dram_tensor("WD", [128, BL * WT_COLS], bf16, kind="ExternalInput")
    wr = nc.dram_tensor("Wr", [128, WT_COLS], bf16, kind="ExternalInput")
    xt = nc.dram_tensor("Xt", [128, BL * 4], bf16, kind="ExternalInput")
    eye = nc.dram_tensor("Eye", [BL, BL], bf16, kind="ExternalInput")
    bias16 = nc.dram_tensor("bias16", [BL, OUT], f32, kind="ExternalInput")
    berr16 = nc.dram_tensor("berr16", [BL, OUT], f32, kind="ExternalInput")
    z = nc.dram_tensor("Z", [1, BL * OUT], f32, kind="ExternalOutput")

    rings = [nc.sync, nc.scalar]

    with tile.TileContext(nc) as tc:
        with (
            tc.tile_pool(name="const", bufs=1) as cpool,
            tc.tile_pool(name="wts", bufs=5) as wpool,
            tc.tile_pool(name="prod", bufs=4) as ptpool,
            tc.tile_pool(name="ps", bufs=8, space="PSUM") as ppool,
        ):
            # ring 0 (SP): slab chunk 0 first, then the small inputs;
            # ring 1 (ACT): slab chunks 1,3,5,... -- so the first slab, W,
            # bias inputs and the second slab all land within ~2us and the
            # engines never sit behind a big serialized stream.
            wt_tiles = {}

            def chunk_dma(ci):
                slabs = CHUNK_SLABS[ci]
                w = len(slabs) * WT_COLS
                # uniform tile size so the pool recycles cleanly
                t = wpool.tile([128, 2 * WT_COLS], bf16, tag="wt")
                ring = rings[1] if ci == len(CHUNK_SLABS) - 1 else rings[ci % 2]
                ring.dma_start(
                    t[:, :w], wd.ap()[:, slabs[0] * WT_COLS : slabs[0] * WT_COLS + w]
                )
                for si, s in enumerate(slabs):
                    wt_tiles[s] = (t, si)

            chunk_dma(0)  # ring0: slab 0
            chunk_dma(1)  # ring1: slab 1
            wr_sb = cpool.tile([128, WT_COLS], bf16)
            nc.sync.dma_start(wr_sb[:], wr.ap())
            xt_sb = cpool.tile([128, BL * 4], bf16)
            nc.sync.dma_start(xt_sb[:], xt.ap())
            eye_sb = cpool.tile([BL, BL], bf16)
            nc.sync.dma_start(eye_sb[:], eye.ap())
            bias_sb = cpool.tile([BL, OUT], f32)
            nc.sync.dma_start(bias_sb[:], bias16.ap())
            berr_sb = cpool.tile([BL, OUT], f32)
            nc.sync.dma_start(berr_sb[:], berr16.ap())
            for ci in range(2, len(CHUNK_SLABS)):
                chunk_dma(ci)

            memb_sb = cpool.tile([BL, OUT], bf16)
            nc.vector.tensor_mul(memb_sb[:], berr_sb[:], bias_sb[:])
            zstage = cpool.tile([1, BL * OUT], f32)

            for b in range(BL):
                wt, si = wt_tiles[b]
                pt = ptpool.tile([128, WT_COLS], bf16, tag="pt")
                mul_eng = nc.gpsimd if b in POOL_SLABS else nc.vector
                mul_eng.tensor_mul(
                    pt[:], wt[:, si * WT_COLS : (si + 1) * WT_COLS], wr_sb[:]
                )
                ps = ppool.tile([1, OUT], f32, tag="ps")
                for j in range(4):
                    nc.tensor.matmul(
                        out=ps[:],
                        lhsT=xt_sb[:, 4 * b + j : 4 * b + j + 1],
                        rhs=pt[:, j * OUT : (j + 1) * OUT],
                        start=(j == 0),
                        stop=False,
                    )
                nc.tensor.matmul(
                    out=ps[:],
                    lhsT=eye_sb[:, b : b + 1],
                    rhs=memb_sb[:],
                    start=False,
                    stop=True,
                )
                nc.scalar.copy(out=zstage[0:1, b * OUT : (b + 1) * OUT], in_=ps[:])
                if b % 4 == 3:
                    g = b // 4
                    nc.scalar.dma_start(
                        z.ap()[:, g * 4 * OUT : (g + 1) * 4 * OUT],
                        zstage[0:1, g * 4 * OUT : (g + 1) * 4 * OUT],
                    )

    nc.compile()
    return nc


def get_nc():
    if "nc" not in _CACHE:
        _CACHE["nc"] = _build()
    return _CACHE["nc"]


def make_in_maps(X, W, bias, Werr, Berr, loc_id):
    X = np.ascontiguousarray(np.asarray(X, dtype=np.float32))
    W = np.ascontiguousarray(np.asarray(W, dtype=np.float32))
    bias = np.ascontiguousarray(np.asarray(bias, dtype=np.float32))
    Werr = np.asarray(Werr, dtype=np.float32)
    Berr = np.asarray(Berr, dtype=np.float32)
    loc_id = np.asarray(loc_id, dtype=np.int32)

    wrb = np.ascontiguousarray(W.reshape(128, WT_COLS).astype(BF16))
    bias16 = np.ascontiguousarray(np.broadcast_to(bias[None, :], (BL, OUT)))
    eye16 = np.eye(BL, dtype=BF16)

    in_maps = []
    for c in range(NCORES):
        xc = X[c * BL : (c + 1) * BL]  # [BL, IN]
        locc = loc_id[c * BL : (c + 1) * BL]  # [BL]
        # slab b in columns [b*2048:(b+1)*2048]; partition p = in-rows 4p..4p+3
        wdc = np.ascontiguousarray(
            Werr[locc]
            .astype(BF16)
            .reshape(BL, 128, WT_COLS)
            .transpose(1, 0, 2)
            .reshape(128, BL * WT_COLS)
        )
        xtc = np.ascontiguousarray(
            xc.reshape(BL, 128, 4).transpose(1, 0, 2).reshape(128, BL * 4).astype(BF16)
        )
        in_maps.append(
            {
                "WD": wdc,
                "Wr": wrb,
                "Xt": xtc,
                "Eye": eye16,
                "bias16": bias16,
                "berr16": np.ascontiguousarray(Berr[locc]),
            }
        )
    return in_maps


def _reset_accelerator():
    import ctypes

    try:
        lib = ctypes.CDLL("/opt/axon/libaxon_pjrt.so")
        lib.axon_reset.restype = ctypes.c_int64
        lib.axon_reset()
    except Exception:
        pass


def kernel(X, W, bias, Werr, Berr, loc_id):
    from concourse.bass_utils import run_bass_kernel_spmd

    nc = get_nc()
    in_maps = make_in_maps(X, W, bias, Werr, Berr, loc_id)
    try:
        res = run_bass_kernel_spmd(nc, in_maps, core_ids=list(range(NCORES)))
    except Exception:
        # a wedged NeuronCore surfaces as an unrecoverable-device error;
        # reset the accelerator once and retry
        _reset_accelerator()
        res = run_bass_kernel_spmd(nc, in_maps, core_ids=list(range(NCORES)))
    out = np.concatenate(
        [res.results[c]["Z"].reshape(BL, OUT) for c in range(NCORES)], axis=0
    )
    return out
